# revision 22
# baseline (speedup 1.0000x reference)
"""Bass/Trainium2 kernel for DenseAtt: out = sigmoid(x@w_i [:,None] + x@w_j [None,:] + b).

Sharding: rows of the (8192, 8192) output are split across 8 NeuronCores
(1024 rows each). Instead of every core loading the full x (4MB) to compute
the column projection b_full = x @ w_j, each core loads only the 2048-row
column SEGMENT containing its own rows (1MB, passed as `xs` with the local
1024 rows first), computes that segment's b directly, and gets the remaining
6144 b values via an AllGather of the per-core (1024,) local projection.
The collective's ~16us latency hides behind the first segment's 8MB of
output stores.

SPMD uniformity (all cores run one program): per-core differences are pure
data —
  * `xs` row order (local rows first; host swaps halves for odd cores),
  * `sel` [4, 3*128]: three one-hot column blocks; matmul with lhsT =
    sel[:, k*128:(k+1)*128] (K=4 segment-partitions) against bf_sb [4, 2048]
    selects global segment g_k AND broadcasts it across 128 partitions in
    one instruction,
  * the host unpermutes each core's output columns when gathering.

Critical-path tricks:
  * b_local for the collective is pb[0:1, 0:1024] — partition 0 of the
    PSUM broadcast tile — DMA'd DRAM-ward directly, so the collective
    launches ~6us in, right after the two local-row matmuls.
  * the linear bias b is accumulated into each [128,1] a-column matmul
    (start/stop PSUM accumulation), so an ACT's bias dep is one column,
    not a whole-tile add.
  * fp32r matmuls (4x full-rate fp32) for the b broadcasts.
  * the self segment sigmoids/stores run at 512-col granularity so stores
    start as soon as chunk 0 is projected.
  * gathered-segment PSUM tiles ping-pong between two 4-bank pools so the
    refill matmuls overlap the previous segment's ACT drain.

The kernel is DMA-bound: 32MB output stores + 1MB xs load per core at
~360GB/s aggregate => ~96.5us busy floor.
"""

import ml_dtypes
import numpy as np

_N = 8192          # rows/cols of the output
_D = 128           # feature dim
_M = 8             # cores
_R = _N // _M      # 1024 rows per core
_CH = 512          # rows per transpose chunk
_SEG = 2048        # output column segment width
_NSEG = _N // _SEG # 4 segments

_nc_cache = None


def _others(c):
    s_c = c // 2
    return [s for s in range(_NSEG) if s != s_c]


def _split_multi_waits(nc, mybir, max_keep=1):
    """Walrus on this toolchain only encodes ONE sem wait per instruction
    (NEURON_ISA_TPB_EVENTS has a single wait slot); Tile emits multi-wait
    sync_info. Split extras onto NoOps inserted right before the instruction
    on the same engine."""
    n_split = 0
    for fn in nc.m.functions:
        for bb in fn.blocks:
            newlist = []
            changed = False
            for inst in list(bb.instructions):
                si = inst.sync_info
                if si is not None and si.on_wait and len(si.on_wait) > max_keep:
                    waits = list(si.on_wait)
                    extra, keep = waits[:-max_keep], waits[-max_keep:]
                    for k, w in enumerate(extra):
                        newlist.append(
                            mybir.InstNoOp(
                                name=f"{inst.name}-waitsplit{k}",
                                engine=inst.engine,
                                sync_info=mybir.SyncInfo(on_wait=[w], on_update=[]),
                                bass_nofuse=True,
                            )
                        )
                        n_split += 1
                    inst.sync_info = mybir.SyncInfo(
                        on_wait=keep, on_update=list(si.on_update)
                    )
                    changed = True
                newlist.append(inst)
            if changed:
                bb.instructions = newlist
    return n_split


def _build():
    global _nc_cache
    if _nc_cache is not None:
        return _nc_cache

    import concourse.bass as bass
    import concourse.mybir as mybir
    from concourse.tile import TileContext

    f32 = mybir.dt.float32
    f32r = mybir.dt.float32r
    bf16 = mybir.dt.bfloat16
    Sigmoid = mybir.ActivationFunctionType.Sigmoid
    Identity = mybir.ActivationFunctionType.Identity

    nc = bass.Bass("TRN2", debug=False, num_devices=_M)

    xs_d = nc.dram_tensor("xs", [_SEG, _D], f32, kind="ExternalInput")
    # packed constants: [:, :128] = eye(128), [:, 128] = w_i, [:, 129] = w_j,
    # [0, 130] = linear bias b
    cst_d = nc.dram_tensor("cst", [_D, _D + 3], f32, kind="ExternalInput")
    # sel[s, k*128+m] = 1.0 iff global segment s is this core's k-th "other"
    # (bf16 to match the gathered-b matmul dtype, cast on the host)
    sel_d = nc.dram_tensor("sel", [_NSEG, 3 * _D], bf16, kind="ExternalInput")
    out_d = nc.dram_tensor("out", [_R, _N], f32, kind="ExternalOutput")

    # row index = t*128 + p  ->  [p, t, d] view for chunked partition loads
    xv = xs_d.ap().rearrange("(t p) d -> p t d", p=128)  # [128, 16, 128]

    with TileContext(nc) as tc:
        with (
            tc.tile_pool(name="const", bufs=1) as cpool,
            tc.tile_pool(name="xin", bufs=4) as xpool,
            tc.tile_pool(name="xt", bufs=4) as xtpool,
            tc.tile_pool(name="outp", bufs=16) as opool,
            tc.tile_pool(name="dram", bufs=2, space="DRAM") as dram,
        ):
            _H = _SEG // 2

            # cst first on the sync HWDGE queue (eye gates the transposes),
            # then all four x chunk loads; sel isn't needed until ~30us so
            # it rides the slow SWDGE queue
            # cst rides SWDGE (desc-gen off the HWDGE critical path); its
            # transfer slots between x0 and x1 on the DMA device
            cst_sb = cpool.tile([128, _D + 3], f32)
            nc.gpsimd.dma_start(out=cst_sb[:], in_=cst_d[:])
            eye_sb = cst_sb[:, 0:_D]
            wi_sb = cst_sb[:, _D:_D + 1]
            wj_sb = cst_sb[:, _D + 1:_D + 2]
            b_sb = cst_sb[0:1, _D + 2:_D + 3]

            x_sbs = []
            for q in range(_SEG // _CH):
                x_sb = xpool.tile([128, _CH // 128, 128], f32, tag="xin",
                                  name=f"x{q}")
                nc.sync.dma_start(out=x_sb[:], in_=xv[:, 4 * q:4 * q + 4, :])
                x_sbs.append(x_sb)

            sel_sb = cpool.tile([_NSEG, 3 * _D], bf16)
            nc.gpsimd.dma_start(out=sel_sb[:], in_=sel_d[:])

            ones_sb = cpool.tile([1, 128], f32)
            nc.vector.memset(ones_sb[:], 1.0)
            zeros_sb = cpool.tile([128, 256], f32)
            nc.vector.memset(zeros_sb[:], 0.0)
            # fp32r matmul operands must be rounded by their producer
            # instruction (walrus verifier), so round copies on DVE
            zeros_r = cpool.tile([128, 256], f32)
            nc.vector.tensor_copy(out=zeros_r[:].bitcast(f32r), in_=zeros_sb[:])
            wj_r = cpool.tile([128, 1], f32)
            nc.vector.tensor_copy(out=wj_r[:].bitcast(f32r), in_=wj_sb)
            # w_j broadcast along free dim: wj_rep[k, m] = w_j[k] for all m
            wj_rep = cpool.tile([128, 128], f32)
            nc.vector.tensor_scalar_add(
                out=wj_rep[:].bitcast(f32r), in0=zeros_sb[:, 0:128],
                scalar1=wj_sb,
            )

            a_raw = cpool.tile([128, _R // 128], f32)
            # the gathered-b path runs in bf16: the ACT copy casts b_local,
            # the collective moves half the bytes, and the sel matmuls run
            # at full PE rate with no extra rounding copies
            bl_sb = cpool.tile([1, _R], bf16)
            bf_sb = cpool.tile([_NSEG, _SEG], bf16)

            bl_d = dram.tile([1, _R], bf16)
            bf_d = dram.tile([_NSEG, _SEG], bf16)

            def sig_store(pb_tile, rt, col0, width):
                o = opool.tile([128, width], f32, tag="o")
                nc.scalar.activation(
                    o[:], pb_tile[:], Sigmoid,
                    bias=a_raw[:, rt:rt + 1], scale=1.0,
                )
                nc.sync.dma_start(
                    out=out_d[rt * 128:(rt + 1) * 128, col0:col0 + width],
                    in_=o[:],
                )

            # ---- self segment ----
            with (
                tc.tile_pool(name="pbA", bufs=2, space="PSUM") as pbA_pool,
                tc.tile_pool(name="pt", bufs=2, space="PSUM") as pt_pool,
                tc.tile_pool(name="pa", bufs=2, space="PSUM") as pa_pool,
            ):
                # PE p-state ramp-up: ~2.5us of back-to-back dummy matmuls so
                # the transposes and projections run at full clock (cold PE
                # is 3.7x slower and everything downstream waits on it)
                warm = pa_pool.tile([128, 256], f32, tag="pa")
                # prow tiles allocated up front: prow1 lands on the warm
                # slot (write-only, no WAR) instead of behind a pa-column
                # copy stuck in the DVE queue
                prows = [
                    pa_pool.tile([1, _CH], f32, tag="pa", name=f"prow{i}")
                    for i in range(2)
                ]
                for _ in range(9):
                    nc.tensor.matmul(
                        warm[:],
                        zeros_r[:, 0:128].bitcast(f32r),
                        zeros_r[:].bitcast(f32r),
                    )

                pbH = [
                    pbA_pool.tile([128, _H], f32, tag="pb", name=f"pbH{i}")
                    for i in range(2)
                ]
                for q in range(_SEG // _CH):
                    pt = pt_pool.tile([128, _CH], f32)
                    for j in range(_CH // 128):
                        nc.tensor.transpose(
                            pt[:, j * 128:(j + 1) * 128], x_sbs[q][:, j, :],
                            eye_sb,
                        )
                    xT = xtpool.tile([128, _CH], f32, tag="xt")
                    nc.vector.tensor_copy(out=xT[:].bitcast(f32r), in_=pt[:])
                    # self segment b, broadcast across partitions (fp32r
                    # runs the fp32 PE at full rate)
                    nc.tensor.matmul(
                        pbH[q // 2][:, (q % 2) * _CH:(q % 2 + 1) * _CH],
                        wj_rep[:].bitcast(f32r),
                        xT[:].bitcast(f32r),
                    )
                    if q < 2:
                        # b_local piece: row-layout projection of this chunk,
                        # copied to SBUF on the ACT engine right away
                        prow = prows[q]
                        with tc.high_priority():
                            nc.tensor.matmul(
                                prow[:], wj_r[:].bitcast(f32r),
                                xT[:].bitcast(f32r),
                            )
                            nc.scalar.activation(
                                bl_sb[:, q * _CH:(q + 1) * _CH], prow[:],
                                Identity,
                            )
                        # local rows: a column per 128-row tile, linear bias
                        # b folded in via PSUM accumulation; high priority so
                        # the DVE copies land before the xT2/xT3 copies and
                        # the sigmoids' bias columns are ready early
                        with tc.high_priority():
                            for r in range(_CH // 128):
                                pa = pa_pool.tile([128, 1], f32, tag="pa")
                                nc.tensor.matmul(
                                    pa[:], xT[:, r * 128:(r + 1) * 128], wi_sb,
                                    start=True, stop=False,
                                )
                                nc.tensor.matmul(
                                    pa[:], ones_sb[:], b_sb,
                                    start=False, stop=True,
                                )
                                rt = q * 4 + r
                                nc.vector.tensor_copy(
                                    out=a_raw[:, rt:rt + 1], in_=pa[:]
                                )
                    if q == 1:
                        # bounce b_local to DRAM on the sync HWDGE queue
                        # (ahead of the store stream) and all-gather b_full
                        with tc.high_priority():
                            nc.sync.dma_start(out=bl_d[:], in_=bl_sb[:])
                        nc.gpsimd.collective_compute(
                            "AllGather",
                            mybir.AluOpType.bypass,
                            replica_groups=[list(range(_M))],
                            ins=[bl_d[:].opt()],
                            outs=[bf_d[:].opt()],
                        )
                        sig_store(pbH[0], 0, 0, _H)

                # remaining self sigmoids + stores; the bf load rides the ACT
                # HWDGE queue just before the last sigmoid (collective is
                # done by then, so no stall, and it slots into the store
                # stream ~5us before the gathered segments need it)
                seq = [(0, rt) for rt in range(1, 8)] + [(1, rt) for rt in range(8)]
                for i, (h, rt) in enumerate(seq):
                    if i == 13:
                        nc.scalar.dma_start(out=bf_sb[:], in_=bf_d[:])
                    sig_store(pbH[h], rt, h * _H, _H)

            # ---- 3 gathered segments: [128,1024] halves in a 4-slot ring ----
            with tc.tile_pool(name="pbB", bufs=4, space="PSUM") as pbB_pool:
                for k in range(3):
                    for hf in range(2):
                        off = k * _SEG + hf * _H  # within bf column space
                        pbk = pbB_pool.tile([128, _H], f32, tag="pb2",
                                            name=f"pbk{k}h{hf}")
                        for j in range(_H // _CH):
                            nc.tensor.matmul(
                                pbk[:, j * _CH:(j + 1) * _CH],
                                sel_sb[:, k * _D:(k + 1) * _D],
                                bf_sb[
                                    :, hf * _H + j * _CH:hf * _H + (j + 1) * _CH
                                ],
                            )
                        for rt in range(_R // 128):
                            sig_store(pbk, rt, (k + 1) * _SEG + hf * _H, _H)

    _split_multi_waits(nc, mybir)

    _nc_cache = nc
    return nc


_runner_cache = None


def _get_runner(nc):
    """Build (once) a jitted shard_map callable around the bass_exec custom
    call, so repeated kernel() calls skip the per-call retrace/recompile that
    run_bass_kernel_spmd's fresh closures would incur."""
    global _runner_cache
    if _runner_cache is not None:
        return _runner_cache

    import jax
    from jax.experimental.shard_map import shard_map
    from jax.sharding import Mesh, PartitionSpec
    from concourse import bass2jax
    import concourse.mybir as mybir

    bass2jax.install_neuronx_cc_hook()

    in_names, out_names, out_avals, zero_outs = [], [], [], []
    for alloc in nc.m.functions[0].allocations:
        if not isinstance(alloc, mybir.MemoryLocationSet):
            continue
        name = alloc.memorylocations[0].name
        if alloc.kind == "ExternalInput":
            in_names.append(name)
        elif alloc.kind == "ExternalOutput":
            out_names.append(name)
            shape = tuple(alloc.tensor_shape)
            dtype = mybir.dt.np(alloc.dtype)
            out_avals.append(jax.core.ShapedArray(shape, dtype))
            zero_outs.append(np.zeros(shape, dtype))

    partition_name = nc.partition_id_tensor.name if nc.partition_id_tensor else None
    if partition_name is not None:
        in_names = [n for n in in_names if n != partition_name]
    n_params = len(in_names)
    all_names = in_names + out_names
    if partition_name is not None:
        all_names = all_names + [partition_name]

    def _body(*args):
        operands = list(args)
        if partition_name is not None:
            operands.append(bass2jax.partition_id_tensor())
        outs = bass2jax._bass_exec_p.bind(
            *operands,
            out_avals=tuple(out_avals),
            in_names=tuple(all_names),
            out_names=tuple(out_names),
            lowering_input_output_aliases=(),
            sim_require_finite=True,
            sim_require_nnan=True,
            nc=nc,
        )
        return tuple(outs)

    devices = jax.devices()[:_M]
    mesh = Mesh(np.asarray(devices), ("core",))
    nspecs = n_params + len(out_names)
    fn = jax.jit(
        shard_map(
            _body,
            mesh=mesh,
            in_specs=(PartitionSpec("core"),) * nspecs,
            out_specs=(PartitionSpec("core"),) * len(out_names),
            check_rep=False,
        ),
        keep_unused=True,
    )
    # Stage the (all-zero) output operands on device once; without donation
    # they are never consumed, so every call reuses them instead of shipping
    # 256MB of zeros through the relay each time.
    from jax.sharding import NamedSharding

    sh = NamedSharding(mesh, PartitionSpec("core"))
    zeros_dev = [
        jax.device_put(np.zeros((_M * z.shape[0], *z.shape[1:]), z.dtype), sh)
        for z in zero_outs
    ]
    _runner_cache = (fn, in_names, zeros_dev)
    return _runner_cache


class _Res:
    exec_time_ns = None
    results = None
    mean_exec_time_ns = None
    instructions_and_trace = None


def _make_in_maps(inputs):
    x = np.ascontiguousarray(np.asarray(inputs["x"], dtype=np.float32))
    w = np.asarray(inputs["w"], dtype=np.float32)
    b = np.asarray(inputs["b"], dtype=np.float32)
    assert x.shape == (_N, _D), x.shape

    cst = np.zeros((_D, _D + 3), dtype=np.float32)
    cst[:, :_D] = np.eye(_D, dtype=np.float32)
    cst[:, _D] = w[0, :_D]
    cst[:, _D + 1] = w[0, _D:]
    cst[0, _D + 2] = b[0]

    maps = []
    for c in range(_M):
        p = c ^ 1
        xs = np.concatenate(
            [x[c * _R:(c + 1) * _R], x[p * _R:(p + 1) * _R]], axis=0
        )
        sel = np.zeros((_NSEG, 3 * _D), dtype=np.float32)
        for k, g in enumerate(_others(c)):
            sel[g, k * _D:(k + 1) * _D] = 1.0
        sel = sel.astype(ml_dtypes.bfloat16)
        maps.append({"xs": np.ascontiguousarray(xs), "cst": cst, "sel": sel})
    return maps


def _gather(blocks):
    """blocks[c] is core c's [1024, 8192] output with columns in
    [self-local, self-partner, g0, g1, g2] segment order; undo the
    permutation into the full [8192, 8192] output."""
    out = np.empty((_N, _N), dtype=np.float32)
    for c, blk in enumerate(blocks):
        p = c ^ 1
        rows = slice(c * _R, (c + 1) * _R)
        out[rows, c * _R:(c + 1) * _R] = blk[:, 0:_R]
        out[rows, p * _R:(p + 1) * _R] = blk[:, _R:2 * _R]
        for k, g in enumerate(_others(c)):
            out[rows, g * _SEG:(g + 1) * _SEG] = blk[
                :, (k + 1) * _SEG:(k + 2) * _SEG
            ]
    return out


def _run(inputs, trace=False, trace_cores=None):
    from concourse._compat import axon_active

    nc = _build()
    in_maps = _make_in_maps(inputs)

    if axon_active() and not trace:
        fn, in_names, zeros_dev = _get_runner(nc)
        args = [
            np.concatenate([m[name] for m in in_maps], axis=0) for name in in_names
        ] + list(zeros_dev)
        out_cat = np.asarray(fn(*args)[0]).reshape(_M, _R, _N)
        return _Res(), _gather(list(out_cat))

    from concourse.bass_utils import run_bass_kernel_spmd

    res = run_bass_kernel_spmd(
        nc, in_maps, core_ids=list(range(_M)), trace=trace, trace_cores=trace_cores
    )
    return res, _gather([r["out"] for r in res.results])


def kernel(**inputs):
    _, out = _run(inputs)
    return out
